# revision 21
# baseline (speedup 1.0000x reference)
"""Multi-head attention (B=2, S=2048, H=16, D=64) on 8 Trainium2 NeuronCores.

Sharding: head-parallel tensor parallelism. Core c owns heads {2c, 2c+1}
(a 128-dim slice of the model dim): column-parallel QKV projections and
local causal attention for its 2 heads, AllToAll of normalized bf16
context (1 MiB/core) pipelined behind attention, then each core runs the
full-width Wo projection for its own disjoint 128-token slices.

Structure (what profiling drove):
 - Attention runs kb-major per (batch, half, head) unit: scores for one
   128-key block against 1024 queries land in a 2-bank PSUM tile, so ONE
   activation call exponentiates [128, <=1024] (96 wide calls instead of
   160 narrow ones; the 352-cycle ACT fixed cost was ~40% of exp time).
 - A*V is flipped: the stationary operand is v [128 keys, 64 dims + ones
   column] and the exp tile streams as the moving operand, so ctx
   accumulates as [dims, queries] in PSUM -- the layout the A2A and Wo
   projection want; no ctx transposes.  PSUM row 64 is the softmax
   denominator for free.
 - Scores run TWO key-blocks ahead of exp, and each unit's first two
   score matmuls are emitted before the previous unit's normalization
   chain, so the scalar engine's exp stream never waits on a unit
   boundary.
 - Normalization avoids the 1-lane DVE reciprocal (7.7 ns/elem on a
   [1,1024] row): the ctx tile is evacuated to SBUF (freeing its PSUM
   immediately), the denominator row is flipped into [128, 8] with 8
   tiny PE transposes, reciprocal'd at full lane parallelism, broadcast
   back with stride-0-stationary matmuls against an fp32 identity, and
   multiplied in.
 - DMA queues: weights lead on SP, x even chunks on SP, x odd chunks on
   the gpsimd software queue, tiny consts on the Activation queue (kept
   otherwise empty so exp is never queued behind a DMA trigger), wo on
   gpsimd behind a tiny warm-up AllToAll that absorbs the collective
   stream's first-op setup.  A2A staging/gathers/output all ride SP:
   software-DGE traffic was observed to triple AllToAll durations.
 - QKV for batch 1 is emitted as single projection chains interleaved
   between batch-0 attention steps; its v transposes (PE+identity) run
   at unit boundaries through the just-freed ctx PSUM slot.
"""

import sys

sys.path.insert(0, "/opt/trn_rl_repo")

import ml_dtypes
import numpy as np

import concourse.bass as bass
import concourse.tile as tile
from concourse import bacc, mybir
from concourse.bass_utils import run_bass_kernel_spmd

N_CORES = 8
B, S, H, D = 2, 2048, 16, 64
E = H * D            # 1024
T = B * S            # 4096 tokens
DPC = 128            # dims (2 heads) per core
NKC = E // 128       # 8 contraction chunks for the projections
NTT = T // 512       # 8 token tiles of 512
NTB = T // 128       # 32 token blocks of 128
PH = 128             # tokens per core per half-batch
QW = 1024            # queries per attention unit (= half batch)

F32 = mybir.dt.float32
BF16 = mybir.dt.bfloat16
AFT = mybir.ActivationFunctionType


def build_program():
    nc = bacc.Bacc("TRN2", target_bir_lowering=False, debug=False,
                   num_devices=N_CORES)

    xT = nc.dram_tensor("xT", [E, T], BF16, kind="ExternalInput").ap()
    # q/k/v weights pre-packed host-side as [sbuf partition, kc, col] so the
    # load is one DMA with 2 KiB descriptors (the [E, DPC] layout would give
    # 256 B descriptors -- measured ~10 GB/s and 35 us of prologue)
    wq2 = nc.dram_tensor("wq2", [128, NKC, DPC], BF16, kind="ExternalInput").ap()
    wk2 = nc.dram_tensor("wk2", [128, NKC, DPC], BF16, kind="ExternalInput").ap()
    wv2 = nc.dram_tensor("wv2", [128, NKC, DPC], BF16, kind="ExternalInput").ap()
    woT = nc.dram_tensor("woT", [E, E], BF16, kind="ExternalInput").ap()
    bqkv = nc.dram_tensor("bqkv", [DPC, 3], F32, kind="ExternalInput").ap()
    bo = nc.dram_tensor("bo", [E], F32, kind="ExternalInput").ap()
    # single 128x128 lower-triangular (k_local <= q_local) mask
    tri = nc.dram_tensor("tri", [128, 128], BF16, kind="ExternalInput").ap()
    ident = nc.dram_tensor("ident", [128, 128], BF16, kind="ExternalInput").ap()
    out = nc.dram_tensor("out", [T // N_CORES, E], F32, kind="ExternalOutput").ap()

    with tile.TileContext(nc) as tc:
        with (
            tc.tile_pool(name="consts", bufs=1) as consts,
            tc.tile_pool(name="state", bufs=1) as state,
            tc.tile_pool(name="ep", bufs=3) as ep,
            tc.tile_pool(name="rp", bufs=2) as rp,
            tc.tile_pool(name="op", bufs=2) as op,
            tc.tile_pool(name="ps_s", bufs=2, space="PSUM") as ps_s,
            tc.tile_pool(name="ps_c", bufs=1, space="PSUM") as ps_c,
            tc.tile_pool(name="ps_m", bufs=2, space="PSUM") as ps_m,
            tc.tile_pool(name="dram", bufs=1, space="DRAM") as dram,
        ):
            # ---- small constants first: the first QKV matmul needs wq,
            # so weights must not sit behind 8 MiB of x in the queues ----
            wq_sb = consts.tile([128, NKC, DPC], BF16)
            wk_sb = consts.tile([128, NKC, DPC], BF16)
            wv_sb = consts.tile([128, NKC, DPC], BF16)
            nc.sync.dma_start(out=wq_sb[:], in_=wq2[:])
            nc.sync.dma_start(out=wk_sb[:], in_=wk2[:])
            nc.sync.dma_start(out=wv_sb[:], in_=wv2[:])
            bqkv_sb = consts.tile([128, 3], F32)
            nc.scalar.dma_start(out=bqkv_sb[:], in_=bqkv[:])
            bq_sb = bqkv_sb[:, 0:1]
            bk_sb = bqkv_sb[:, 1:2]
            bv_sb = bqkv_sb[:, 2:3]
            bo_bc = consts.tile([128, E], F32)
            nc.scalar.dma_start(
                out=bo_bc[:],
                in_=bass.AP(tensor=bo.tensor, offset=bo.offset,
                            ap=[[0, 128], [1, E]]),
            )
            tri_sb = consts.tile([128, 128], BF16)
            nc.scalar.dma_start(out=tri_sb[:], in_=tri[:])
            id_sb = consts.tile([128, 128], BF16)
            nc.scalar.dma_start(out=id_sb[:], in_=ident[:])
            idf_sb = consts.tile([128, 128], F32)
            nc.vector.tensor_copy(idf_sb[:], id_sb[:])
            onef = consts.tile([128, 1], F32)
            nc.vector.memset(onef[:], 1.0)

            # tiny warm-up AllToAll on the gpsimd queue: absorbs the
            # collective stream's first-op setup while QKV runs
            ctxw = dram.tile([N_CORES, 128, 2], BF16, tag="ctxw", name="ctxw")
            recvw = dram.tile([N_CORES, 128, 2], BF16, tag="recvw",
                              name="recvw")
            nc.gpsimd.dma_start(out=ctxw[:], in_=tri[:, 0:16])
            nc.gpsimd.collective_compute(
                "AllToAll", mybir.AluOpType.bypass,
                replica_groups=[list(range(N_CORES))],
                ins=[ctxw.opt()], outs=[recvw.opt()],
            )

            # ---- x: a small fast-start chunk per kc so QKV tile 0 can
            # begin, then wide descriptors for bandwidth; batch-1 halves
            # alternate between SP and the gpsimd software queue ----
            x_sb = state.tile([128, NKC, T], BF16)  # full x^T in SBUF

            def x_load(eng, kc, c0, c1):
                eng.dma_start(out=x_sb[:, kc, c0:c1],
                              in_=xT[kc * 128:(kc + 1) * 128, c0:c1])

            engs3 = [nc.sync, nc.scalar, nc.gpsimd]
            for kc in range(NKC):
                x_load(engs3[kc % 3], kc, 0, 1024)
            for kc in range(NKC):
                x_load(engs3[(kc + 1) % 3], kc, 1024, 2048)
            for kc in range(NKC):
                x_load(engs3[(kc + 2) % 3], kc, 2048, 3072)
            for kc in range(NKC):
                x_load(engs3[kc % 3], kc, 3072, 4096)

            # wo on the gpsimd software queue behind x-odds
            wo_sb = consts.tile([128, NKC, E], BF16)
            for kc in range(NKC):
                nc.gpsimd.dma_start(out=wo_sb[:, kc, :],
                                    in_=woT[kc * 128:(kc + 1) * 128, :])

            # ---- persistent activations -----------------------------------
            qT_sb = state.tile([128, T], BF16)   # [2-head dims, tokens]
            # per-head k^T zero-padded to the full 128 partitions: head h
            # occupies partitions [64h, 64h+64), the rest stay zero.
            kTp = [state.tile([128, T], BF16, name=f"kTp{h}") for h in range(2)]
            # v in [token, dim] layout per 128-token block:
            # cols 0:64 = head0 v, 64:66 = [1, 0], 66:130 = head1 v,
            # 130:132 = [1, 0]. The [1,0] columns give each head's AV
            # stationary slice (0:66 / 66:132) a softmax-denominator row.
            vN_sb = state.tile([128, NTB, 132], BF16)
            ctxT_sb = state.tile([128, T], BF16)  # normalized ctx, [dims, tok]

            nc.vector.memset(kTp[0][64:128, :], 0.0)
            nc.vector.memset(kTp[1][0:64, :], 0.0)
            nc.vector.memset(vN_sb[:, :, 64:65], 1.0)
            nc.vector.memset(vN_sb[:, :, 65:66], 0.0)
            nc.vector.memset(vN_sb[:, :, 130:131], 1.0)
            nc.vector.memset(vN_sb[:, :, 131:132], 0.0)

            # ---- QKV projection emitters ----------------------------------
            # epilogues on DVE; v's [token, dim] reshape via PE transposes
            # whose PSUM rides the ps_c slot (free between attention units).
            def emit_v_transpose_one(tt, vt_sb, i):
                tb = tt * 4 + i
                tp_ps = ps_c.tile([128, 128], BF16, tag="c", name="tp_ps")
                nc.tensor.transpose(
                    tp_ps[:], vt_sb[:, i * 128:(i + 1) * 128], id_sb[:])
                nc.vector.tensor_copy(vN_sb[:, tb, 0:64], tp_ps[:, 0:64])
                nc.vector.tensor_copy(vN_sb[:, tb, 66:130], tp_ps[:, 64:128])

            def emit_v_transposes(tt, vt_sb):
                for i in range(4):
                    emit_v_transpose_one(tt, vt_sb, i)

            def emit_qkv_chain(tt, which, halves=(0, 1), ps_box=[None]):
                ts = slice(tt * 512, (tt + 1) * 512)
                w_sb, b_sb = {"q": (wq_sb, bq_sb), "k": (wk_sb, bk_sb),
                              "v": (wv_sb, bv_sb)}[which]
                if 0 in halves:
                    ps_box[0] = ps_m.tile([128, 512], F32, tag="m", name="ps")
                ps = ps_box[0]
                for hh in halves:
                    for kc in range(hh * 4, hh * 4 + 4):
                        nc.tensor.matmul(ps[:], w_sb[:, kc, :],
                                         x_sb[:, kc, ts],
                                         start=(kc == 0),
                                         stop=(kc == NKC - 1))
                if 1 not in halves:
                    return None
                if which == "q":
                    nc.vector.tensor_scalar_add(qT_sb[:, ts], ps[:], b_sb[:])
                elif which == "k":
                    nc.vector.tensor_scalar_add(
                        kTp[0][0:64, ts], ps[0:64, :], b_sb[0:64, :])
                    nc.vector.tensor_scalar_add(
                        kTp[1][64:128, ts], ps[64:128, :], b_sb[64:128, :])
                else:
                    vt_sb = rp.tile([128, 512], BF16, tag="vt", name="vt",
                                    bufs=4)
                    nc.vector.tensor_scalar_add(vt_sb[:], ps[:], b_sb[:])
                    return vt_sb
                return None

            def emit_qkv_tile(tt):
                emit_qkv_chain(tt, "q")
                emit_qkv_chain(tt, "k")
                vt_sb = emit_qkv_chain(tt, "v")
                emit_v_transposes(tt, vt_sb)

            # deferred QKV work (tiles 2..7): half-chains between attention
            # steps; v transposes wait for a unit boundary where the freed
            # ctx PSUM slot can host them (tiles 2-3 feed the second
            # batch-0 half, 4-7 must all be in place before batch 1)
            filler = [(tt, w, hh) for tt in range(2, NTT)
                      for w in ("q", "k", "v") for hh in (0, 1)]
            fill_box = {}
            deferred_tr = []
            evac_pops = [1, 1, 2, 2, 0, 0, 0, 0]
            evac_idx = [0]

            def maybe_fill():
                if filler:
                    tt, w, hh = filler.pop(0)
                    box = fill_box.setdefault((tt, w), [None])
                    vt_sb = emit_qkv_chain(tt, w, halves=(hh,), ps_box=box)
                    if vt_sb is not None:
                        deferred_tr.append((tt, vt_sb))

            # ---- attention unit: (batch b, half hf, head h) ---------------
            # kb-major: scores for each 128-key block land in a [128, 1024]
            # PSUM tile (two ahead of exp), one exp call per block, A*V
            # accumulates ctx [66, 1024].  The previous unit's finisher is
            # split in two: evac (frees its ctx PSUM slot; must precede this
            # unit's ctx allocation) runs after this unit's first two score
            # matmuls, and the reciprocal/broadcast/normalize chain runs one
            # kb-step into this unit's loop -- so exp never waits.
            def emit_attention_unit(b, hf, h, prev_evac, prev_rest,
                                    hook=None):
                t0 = b * S
                qb0 = hf * QW                  # query base within batch
                q0 = t0 + qb0                  # query base global
                nkb = (qb0 + QW) // 128        # key blocks: 8 or 16
                d0 = h * 64

                def emit_scores(kb):
                    c_lo = max(0, 128 * kb - qb0)
                    s_ps = ps_s.tile([128, QW], F32, tag="s", name="s_ps")
                    for s0 in (0, 512):
                        lo = max(c_lo, s0)
                        if lo < s0 + 512:
                            nc.tensor.matmul(
                                s_ps[:, lo:s0 + 512],
                                kTp[h][:, t0 + kb * 128:t0 + (kb + 1) * 128],
                                qT_sb[:, q0 + lo:q0 + s0 + 512],
                                start=True, stop=True)
                    return s_ps, c_lo

                s_tiles = {0: emit_scores(0)}
                if nkb > 1:
                    s_tiles[1] = emit_scores(1)
                if prev_evac is not None:
                    prev_evac()
                cn_ps = ps_c.tile([128, QW], F32, tag="c", name="cn_ps")
                for kb in range(nkb):
                    if kb + 2 < nkb:
                        s_tiles[kb + 2] = emit_scores(kb + 2)
                    s_ps, c_lo = s_tiles.pop(kb)
                    w = QW - c_lo
                    e_sb = ep.tile([128, QW], BF16, tag="e", name="e_sb")
                    nc.scalar.activation(e_sb[:, 0:w], s_ps[:, c_lo:QW],
                                         AFT.Exp, scale=0.125)
                    dcol = 128 * kb - qb0
                    if dcol >= 0:  # diagonal block: in-block causal mask
                        nc.vector.tensor_mul(e_sb[:, 0:128],
                                             e_sb[:, 0:128], tri_sb[:])
                    for s0 in (0, 512):
                        lo = max(c_lo, s0)
                        if lo < s0 + 512:
                            nc.tensor.matmul(
                                cn_ps[0:66, lo:s0 + 512],
                                vN_sb[:, b * (S // 128) + kb,
                                      h * 66:(h + 1) * 66],
                                e_sb[:, lo - c_lo:s0 + 512 - c_lo],
                                start=(kb == 0), stop=(kb == nkb - 1),
                                skip_group_check=True)
                    if kb == 0 and prev_rest is not None:
                        prev_rest()
                    if kb == 1 and hook is not None:
                        hook()
                    maybe_fill()

                ct = rp.tile([128, QW], F32, tag="ct", name="ct")

                def evac():
                    # evacuate ctx+denominator to SBUF, freeing cn_ps; the
                    # scheduled batch-1 v transposes then ride the free slot
                    nc.vector.tensor_copy(ct[0:66, :], cn_ps[0:66, :])
                    n = evac_pops[evac_idx[0]] if evac_idx[0] < 8 else 0
                    evac_idx[0] += 1
                    for _ in range(n):
                        if deferred_tr:
                            emit_v_transposes(*deferred_tr.pop(0))

                def rest():
                    # flip the denominator row into [128, 8], reciprocal at
                    # full lane parallelism, broadcast back via stride-0
                    # stationary matmuls against the fp32 identity, multiply.
                    T8 = ps_m.tile([128, 8], F32, tag="m", name="T8")
                    for j in range(8):
                        nc.tensor.transpose(
                            T8[:, j:j + 1], ct[64:65, j * 128:(j + 1) * 128],
                            onef[64:65, :])
                    R8 = rp.tile([128, 8], F32, tag="r8", name="R8")
                    nc.vector.reciprocal(R8[:], T8[:])
                    for half in range(2):
                        bc = ps_m.tile([64, 512], F32, tag="m", name="bc")
                        for jj in range(4):
                            j = half * 4 + jj
                            col = R8[:, j:j + 1]
                            lhsT = bass.AP(tensor=col.tensor, offset=col.offset,
                                           ap=[col.ap[0], [0, 64]])
                            nc.tensor.matmul(
                                bc[0:64, jj * 128:(jj + 1) * 128], lhsT,
                                idf_sb[:], start=True, stop=True)
                        sg = slice(half * 512, (half + 1) * 512)
                        nc.vector.tensor_mul(
                            ctxT_sb[d0:d0 + 64, q0 + half * 512:
                                    q0 + (half + 1) * 512],
                            ct[0:64, sg], bc[0:64, :])

                return evac, rest

            # ---- A2A + local Wo projection --------------------------------
            def emit_half_a2a_head(b, hf, h):
                # half-payload A2A carrying one head's 64 ctx rows; used to
                # overlap most of the final half's exchange with its last
                # attention unit
                base = b * S + hf * (S // 2)
                r0 = h * 64
                ctxd = dram.tile([N_CORES, 64, PH], BF16, tag=f"ctxdh{h}",
                                 name="ctxdh", bufs=1)
                for j in range(N_CORES):
                    nc.sync.dma_start(
                        out=ctxd[j],
                        in_=ctxT_sb[r0:r0 + 64,
                                    base + j * PH:base + (j + 1) * PH])
                recv = dram.tile([N_CORES, 64, PH], BF16, tag=f"recvh{h}",
                                 name="recvh", bufs=1)
                nc.gpsimd.collective_compute(
                    "AllToAll", mybir.AluOpType.bypass,
                    replica_groups=[list(range(N_CORES))],
                    ins=[ctxd.opt()], outs=[recv.opt()],
                )
                return recv

            def emit_half_a2a(b, hf):
                base = b * S + hf * (S // 2)
                ctxd = dram.tile([N_CORES, 128, PH], BF16, tag="ctxd",
                                 name="ctxd", bufs=4)
                for j in range(N_CORES):
                    nc.sync.dma_start(
                        out=ctxd[j],
                        in_=ctxT_sb[:, base + j * PH:base + (j + 1) * PH])
                recv = dram.tile([N_CORES, 128, PH], BF16, tag="recv",
                                 name="recv", bufs=4)
                nc.gpsimd.collective_compute(
                    "AllToAll",
                    mybir.AluOpType.bypass,
                    replica_groups=[list(range(N_CORES))],
                    ins=[ctxd.opt()],
                    outs=[recv.opt()],
                )
                return recv

            def emit_half_gather(b, hf, recv):
                cg_sb = op.tile([128, NKC, PH], BF16, tag="cg_sb", name="cg_sb")
                for j in range(N_CORES):
                    nc.sync.dma_start(out=cg_sb[:, j, :], in_=recv[j])
                return cg_sb

            def emit_half_proj(b, hf, cg_sb):
                o_sb = op.tile([PH, E], F32, tag="o_sb", name="o_sb")
                for et in range(2):
                    ps = ps_m.tile([128, 512], F32, tag="m", name="c_ps")
                    for kc in range(NKC):
                        nc.tensor.matmul(
                            ps[0:PH, :],
                            cg_sb[:, kc, :],
                            wo_sb[:, kc, et * 512:(et + 1) * 512],
                            start=(kc == 0), stop=(kc == NKC - 1))
                    nc.vector.tensor_add(
                        o_sb[:, et * 512:(et + 1) * 512], ps[0:PH, :],
                        bo_bc[0:PH, et * 512:(et + 1) * 512])
                r0 = (b * 2 + hf) * PH
                nc.sync.dma_start(out=out[r0:r0 + PH, :], in_=o_sb[:])

            # ---- main schedule --------------------------------------------
            # only tokens 0-1023 are needed before the first attention unit
            for tt in range(2):
                emit_qkv_tile(tt)

            sent = []      # (b, hf, recv): a2a issued, gather not emitted
            gathered = []  # (b, hf, cg_sb): gather emitted, proj not
            a2a_req = None  # (b, hf) whose ctx is complete, a2a not issued
            prev_evac = prev_rest = None
            # batch 1 processes hf1 before hf0 so the final (serial) A2A +
            # projection covers the cheap 8-key-block half.
            order = [(0, 0), (0, 1), (1, 1), (1, 0)]
            recvh0_box = []
            for b, hf in order:
                last_half = (b, hf) == order[-1]
                for h in range(2):
                    hook = None
                    if last_half and h == 1:
                        # fire the final half's head-0 A2A as soon as its
                        # normalization (run in this unit's prologue) lands
                        hook = (lambda: recvh0_box.append(
                            emit_half_a2a_head(b, hf, 0)))
                    prev_evac, prev_rest = emit_attention_unit(
                        b, hf, h, prev_evac, prev_rest, hook=hook)
                    # advance the a2a -> gather -> projection pipeline one
                    # stage per unit so each step overlaps attention compute
                    if gathered:
                        emit_half_proj(*gathered.pop(0))
                    if sent:
                        bb, hh2, recv = sent.pop(0)
                        gathered.append(
                            (bb, hh2, emit_half_gather(bb, hh2, recv)))
                    if a2a_req is not None:
                        # previous half's ctx is fully written (its norm ran
                        # inside this unit's prologue)
                        sent.append((a2a_req[0], a2a_req[1],
                                     emit_half_a2a(*a2a_req)))
                        a2a_req = None
                if not last_half:
                    a2a_req = (b, hf)
            prev_evac()
            prev_rest()
            while filler:
                tt, w, hh = filler.pop(0)
                box = fill_box.setdefault((tt, w), [None])
                vt_sb = emit_qkv_chain(tt, w, halves=(hh,), ps_box=box)
                if vt_sb is not None:
                    deferred_tr.append((tt, vt_sb))
            while deferred_tr:
                emit_v_transposes(*deferred_tr.pop(0))
            while sent:
                bb, hh2, recv = sent.pop(0)
                gathered.append((bb, hh2, emit_half_gather(bb, hh2, recv)))
            while gathered:
                emit_half_proj(*gathered.pop(0))
            # final half: head-1 A2A now, head-0 was fired mid-attention
            bL, hfL = order[-1]
            recvh1 = emit_half_a2a_head(bL, hfL, 1)
            cg_sb = op.tile([128, NKC, PH], BF16, tag="cg_sb", name="cg_sb")
            for j in range(N_CORES):
                nc.sync.dma_start(out=cg_sb[0:64, j, :], in_=recvh0_box[0][j])
                nc.sync.dma_start(out=cg_sb[64:128, j, :], in_=recvh1[j])
            emit_half_proj(bL, hfL, cg_sb)

    nc.compile()
    return nc


_NC = None


def _get_program():
    global _NC
    if _NC is None:
        _NC = build_program()
    return _NC


def _bf(a):
    return np.ascontiguousarray(a).astype(ml_dtypes.bfloat16)


def kernel(x, Wq, bq, Wk, bk, Wv, bv, Wo, bo, _trace=False, _trace_kwargs=None):
    x = np.asarray(x, np.float32)
    Wq, Wk, Wv, Wo = (np.asarray(w, np.float32) for w in (Wq, Wk, Wv, Wo))
    bq, bk, bv, bo = (np.asarray(v, np.float32) for v in (bq, bk, bv, bo))

    xT = _bf(x.reshape(T, E).T)
    i = np.arange(128)
    tri = _bf((i[:, None] <= i[None, :]).astype(np.float32))
    ident = _bf(np.eye(128, dtype=np.float32))

    def _packw(w_slice):
        # [DPC, E] weight slice -> [128, kc, 128]: p-major rows so the SBUF
        # load is one DMA with 2 KiB per-partition descriptors
        return _bf(np.ascontiguousarray(
            w_slice.T.reshape(NKC, 128, DPC).transpose(1, 0, 2)))

    in_maps = []
    for c in range(N_CORES):
        sl = slice(c * DPC, (c + 1) * DPC)
        in_maps.append({
            "xT": xT,
            "wq2": _packw(Wq[sl, :]),
            "wk2": _packw(Wk[sl, :]),
            "wv2": _packw(Wv[sl, :]),
            "woT": _bf(Wo.T),
            "bqkv": np.ascontiguousarray(
                np.stack([bq[sl], bk[sl], bv[sl]], axis=1).astype(np.float32)),
            "bo": bo,
            "tri": tri,
            "ident": ident,
        })

    nc = _get_program()
    res = run_bass_kernel_spmd(nc, in_maps, list(range(N_CORES)),
                               trace=_trace, **(_trace_kwargs or {}))
    # out[c] rows are [batch, half, 128]: row (b, hf, r) holds global
    # token b*2048 + hf*1024 + c*128 + r.
    stacked = np.stack([res.results[i]["out"].reshape(B, 2, 128, E)
                        for i in range(N_CORES)], axis=2)
    full = stacked.reshape(T, E)
    if _trace:
        return full.reshape(B, S, E), res
    return full.reshape(B, S, E)


# revision 22
# speedup vs baseline: 1.0477x; 1.0477x over previous
"""Multi-head attention (B=2, S=2048, H=16, D=64) on 8 Trainium2 NeuronCores.

Sharding: head-parallel tensor parallelism. Core c owns heads {2c, 2c+1}
(a 128-dim slice of the model dim): column-parallel QKV projections and
local causal attention for its 2 heads, AllToAll of normalized bf16
context (1 MiB/core) pipelined behind attention, then each core runs the
full-width Wo projection for its own disjoint 128-token slices.

Structure (what profiling drove):
 - Attention runs kb-major per (batch, half, head) unit: scores for one
   128-key block against 1024 queries land in a 2-bank PSUM tile, so ONE
   activation call exponentiates [128, <=1024] (96 wide calls instead of
   160 narrow ones; the 352-cycle ACT fixed cost was ~40% of exp time).
 - A*V is flipped: the stationary operand is v [128 keys, 64 dims + ones
   column] and the exp tile streams as the moving operand, so ctx
   accumulates as [dims, queries] in PSUM -- the layout the A2A and Wo
   projection want; no ctx transposes.  PSUM row 64 is the softmax
   denominator for free.
 - Scores run TWO key-blocks ahead of exp, and each unit's first two
   score matmuls are emitted before the previous unit's normalization
   chain, so the scalar engine's exp stream never waits on a unit
   boundary.
 - Normalization avoids the 1-lane DVE reciprocal (7.7 ns/elem on a
   [1,1024] row): the ctx tile is evacuated to SBUF (freeing its PSUM
   immediately), the denominator row is flipped into [128, 8] with 8
   tiny PE transposes, reciprocal'd at full lane parallelism, broadcast
   back with stride-0-stationary matmuls against an fp32 identity, and
   multiplied in.
 - DMA queues: weights lead on SP, x even chunks on SP, x odd chunks on
   the gpsimd software queue, tiny consts on the Activation queue (kept
   otherwise empty so exp is never queued behind a DMA trigger), wo on
   gpsimd behind a tiny warm-up AllToAll that absorbs the collective
   stream's first-op setup.  A2A staging/gathers/output all ride SP:
   software-DGE traffic was observed to triple AllToAll durations.
 - QKV for batch 1 is emitted as single projection chains interleaved
   between batch-0 attention steps; its v transposes (PE+identity) run
   at unit boundaries through the just-freed ctx PSUM slot.
"""

import sys

sys.path.insert(0, "/opt/trn_rl_repo")

import ml_dtypes
import numpy as np

import concourse.bass as bass
import concourse.tile as tile
from concourse import bacc, mybir
from concourse.bass_utils import run_bass_kernel_spmd

N_CORES = 8
B, S, H, D = 2, 2048, 16, 64
E = H * D            # 1024
T = B * S            # 4096 tokens
DPC = 128            # dims (2 heads) per core
NKC = E // 128       # 8 contraction chunks for the projections
NTT = T // 512       # 8 token tiles of 512
NTB = T // 128       # 32 token blocks of 128
PH = 128             # tokens per core per half-batch
QW = 1024            # queries per attention unit (= half batch)

F32 = mybir.dt.float32
BF16 = mybir.dt.bfloat16
AFT = mybir.ActivationFunctionType


def build_program():
    nc = bacc.Bacc("TRN2", target_bir_lowering=False, debug=False,
                   num_devices=N_CORES)

    xT = nc.dram_tensor("xT", [E, T], BF16, kind="ExternalInput").ap()
    # q/k/v weights pre-packed host-side as [sbuf partition, kc, col] so the
    # load is one DMA with 2 KiB descriptors (the [E, DPC] layout would give
    # 256 B descriptors -- measured ~10 GB/s and 35 us of prologue)
    wq2 = nc.dram_tensor("wq2", [128, NKC, DPC], BF16, kind="ExternalInput").ap()
    wk2 = nc.dram_tensor("wk2", [128, NKC, DPC], BF16, kind="ExternalInput").ap()
    wv2 = nc.dram_tensor("wv2", [128, NKC, DPC], BF16, kind="ExternalInput").ap()
    woT = nc.dram_tensor("woT", [E, E], BF16, kind="ExternalInput").ap()
    bqkv = nc.dram_tensor("bqkv", [DPC, 3], F32, kind="ExternalInput").ap()
    bo = nc.dram_tensor("bo", [E], F32, kind="ExternalInput").ap()
    # single 128x128 lower-triangular (k_local <= q_local) mask
    tri = nc.dram_tensor("tri", [128, 128], BF16, kind="ExternalInput").ap()
    ident = nc.dram_tensor("ident", [128, 128], BF16, kind="ExternalInput").ap()
    out = nc.dram_tensor("out", [T // N_CORES, E], F32, kind="ExternalOutput").ap()

    with tile.TileContext(nc) as tc:
        with (
            tc.tile_pool(name="consts", bufs=1) as consts,
            tc.tile_pool(name="state", bufs=1) as state,
            tc.tile_pool(name="ep", bufs=3) as ep,
            tc.tile_pool(name="rp", bufs=2) as rp,
            tc.tile_pool(name="op", bufs=2) as op,
            tc.tile_pool(name="ps_s", bufs=2, space="PSUM") as ps_s,
            tc.tile_pool(name="ps_c", bufs=1, space="PSUM") as ps_c,
            tc.tile_pool(name="ps_m", bufs=2, space="PSUM") as ps_m,
            tc.tile_pool(name="dram", bufs=1, space="DRAM") as dram,
        ):
            # ---- small constants first: the first QKV matmul needs wq,
            # so weights must not sit behind 8 MiB of x in the queues ----
            wq_sb = consts.tile([128, NKC, DPC], BF16)
            wk_sb = consts.tile([128, NKC, DPC], BF16)
            wv_sb = consts.tile([128, NKC, DPC], BF16)
            nc.sync.dma_start(out=wq_sb[:], in_=wq2[:])
            nc.sync.dma_start(out=wk_sb[:], in_=wk2[:])
            nc.sync.dma_start(out=wv_sb[:], in_=wv2[:])
            bqkv_sb = consts.tile([128, 3], F32)
            nc.scalar.dma_start(out=bqkv_sb[:], in_=bqkv[:])
            bq_sb = bqkv_sb[:, 0:1]
            bk_sb = bqkv_sb[:, 1:2]
            bv_sb = bqkv_sb[:, 2:3]
            bo_bc = consts.tile([128, E], F32)
            nc.scalar.dma_start(
                out=bo_bc[:],
                in_=bass.AP(tensor=bo.tensor, offset=bo.offset,
                            ap=[[0, 128], [1, E]]),
            )
            tri_sb = consts.tile([128, 128], BF16)
            nc.scalar.dma_start(out=tri_sb[:], in_=tri[:])
            id_sb = consts.tile([128, 128], BF16)
            nc.scalar.dma_start(out=id_sb[:], in_=ident[:])
            idf_sb = consts.tile([128, 128], F32)
            nc.vector.tensor_copy(idf_sb[:], id_sb[:])
            onef = consts.tile([128, 1], F32)
            nc.vector.memset(onef[:], 1.0)

            # tiny warm-up AllToAll on the gpsimd queue: absorbs the
            # collective stream's first-op setup while QKV runs
            ctxw = dram.tile([N_CORES, 128, 2], BF16, tag="ctxw", name="ctxw")
            recvw = dram.tile([N_CORES, 128, 2], BF16, tag="recvw",
                              name="recvw")
            nc.gpsimd.dma_start(out=ctxw[:], in_=tri[:, 0:16])
            nc.gpsimd.collective_compute(
                "AllToAll", mybir.AluOpType.bypass,
                replica_groups=[list(range(N_CORES))],
                ins=[ctxw.opt()], outs=[recvw.opt()],
            )

            # ---- x: a small fast-start chunk per kc so QKV tile 0 can
            # begin, then wide descriptors for bandwidth; batch-1 halves
            # alternate between SP and the gpsimd software queue ----
            x_sb = state.tile([128, NKC, T], BF16)  # full x^T in SBUF

            def x_load(eng, kc, c0, c1):
                eng.dma_start(out=x_sb[:, kc, c0:c1],
                              in_=xT[kc * 128:(kc + 1) * 128, c0:c1])

            engs3 = [nc.sync, nc.scalar, nc.gpsimd]
            for kc in range(NKC):
                x_load(engs3[kc % 3], kc, 0, 1024)
            for kc in range(NKC):
                x_load(engs3[(kc + 1) % 3], kc, 1024, 2048)
            for kc in range(NKC):
                x_load(engs3[(kc + 2) % 3], kc, 2048, 3072)
            for kc in range(NKC):
                x_load(engs3[kc % 3], kc, 3072, 4096)

            # wo on the gpsimd software queue behind x-odds
            wo_sb = consts.tile([128, NKC, E], BF16)
            for kc in range(NKC):
                nc.gpsimd.dma_start(out=wo_sb[:, kc, :],
                                    in_=woT[kc * 128:(kc + 1) * 128, :])

            # ---- persistent activations -----------------------------------
            qT_sb = state.tile([128, T], BF16)   # [2-head dims, tokens]
            # per-head k^T zero-padded to the full 128 partitions: head h
            # occupies partitions [64h, 64h+64), the rest stay zero.
            kTp = [state.tile([128, T], BF16, name=f"kTp{h}") for h in range(2)]
            # v in [token, dim] layout per 128-token block:
            # cols 0:64 = head0 v, 64:66 = [1, 0], 66:130 = head1 v,
            # 130:132 = [1, 0]. The [1,0] columns give each head's AV
            # stationary slice (0:66 / 66:132) a softmax-denominator row.
            vN_sb = state.tile([128, NTB, 132], BF16)
            ctxT_sb = state.tile([128, T], BF16)  # normalized ctx, [dims, tok]

            nc.vector.memset(kTp[0][64:128, :], 0.0)
            nc.vector.memset(kTp[1][0:64, :], 0.0)
            nc.vector.memset(vN_sb[:, :, 64:65], 1.0)
            nc.vector.memset(vN_sb[:, :, 65:66], 0.0)
            nc.vector.memset(vN_sb[:, :, 130:131], 1.0)
            nc.vector.memset(vN_sb[:, :, 131:132], 0.0)

            # ---- QKV projection emitters ----------------------------------
            # epilogues on DVE; v's [token, dim] reshape via PE transposes
            # whose PSUM rides the ps_c slot (free between attention units).
            def emit_v_transpose_one(tt, vt_sb, i):
                tb = tt * 4 + i
                tp_ps = ps_c.tile([128, 128], BF16, tag="c", name="tp_ps")
                nc.tensor.transpose(
                    tp_ps[:], vt_sb[:, i * 128:(i + 1) * 128], id_sb[:])
                nc.vector.tensor_copy(vN_sb[:, tb, 0:64], tp_ps[:, 0:64])
                nc.vector.tensor_copy(vN_sb[:, tb, 66:130], tp_ps[:, 64:128])

            def emit_v_transposes(tt, vt_sb):
                for i in range(4):
                    emit_v_transpose_one(tt, vt_sb, i)

            def emit_qkv_chain(tt, which, halves=(0, 1), ps_box=[None]):
                ts = slice(tt * 512, (tt + 1) * 512)
                w_sb, b_sb = {"q": (wq_sb, bq_sb), "k": (wk_sb, bk_sb),
                              "v": (wv_sb, bv_sb)}[which]
                if 0 in halves:
                    ps_box[0] = ps_m.tile([128, 512], F32, tag="m", name="ps")
                ps = ps_box[0]
                for hh in halves:
                    for kc in range(hh * 4, hh * 4 + 4):
                        nc.tensor.matmul(ps[:], w_sb[:, kc, :],
                                         x_sb[:, kc, ts],
                                         start=(kc == 0),
                                         stop=(kc == NKC - 1))
                if 1 not in halves:
                    return None
                if which == "q":
                    nc.vector.tensor_scalar_add(qT_sb[:, ts], ps[:], b_sb[:])
                elif which == "k":
                    nc.vector.tensor_scalar_add(
                        kTp[0][0:64, ts], ps[0:64, :], b_sb[0:64, :])
                    nc.vector.tensor_scalar_add(
                        kTp[1][64:128, ts], ps[64:128, :], b_sb[64:128, :])
                else:
                    vt_sb = rp.tile([128, 512], BF16, tag="vt", name="vt",
                                    bufs=4)
                    nc.vector.tensor_scalar_add(vt_sb[:], ps[:], b_sb[:])
                    return vt_sb
                return None

            def emit_qkv_tile(tt):
                emit_qkv_chain(tt, "q")
                emit_qkv_chain(tt, "k")
                vt_sb = emit_qkv_chain(tt, "v")
                emit_v_transposes(tt, vt_sb)

            # deferred QKV work (tiles 2..7): half-chains between attention
            # steps; v transposes wait for a unit boundary where the freed
            # ctx PSUM slot can host them (tiles 2-3 feed the second
            # batch-0 half, 4-7 must all be in place before batch 1)
            filler = [(tt, w, hh) for tt in range(2, NTT)
                      for w in ("q", "k", "v") for hh in (0, 1)]
            fill_box = {}
            deferred_tr = []
            step_count = [0]

            def maybe_fill():
                # skip the first 8 attention steps: x for tiles 2+ is still
                # in flight then, and a stalled filler matmul would block
                # the whole PE queue (strict FIFO) and starve exp
                step_count[0] += 1
                if filler and step_count[0] > 8:
                    tt, w, hh = filler.pop(0)
                    box = fill_box.setdefault((tt, w), [None])
                    vt_sb = emit_qkv_chain(tt, w, halves=(hh,), ps_box=box)
                    if vt_sb is not None:
                        deferred_tr.append((tt, vt_sb))

            # ---- attention unit: (batch b, half hf, head h) ---------------
            # kb-major: scores for each 128-key block land in a [128, 1024]
            # PSUM tile (two ahead of exp), one exp call per block, A*V
            # accumulates ctx [66, 1024].  The previous unit's finisher is
            # split in two: evac (frees its ctx PSUM slot; must precede this
            # unit's ctx allocation) runs after this unit's first two score
            # matmuls, and the reciprocal/broadcast/normalize chain runs one
            # kb-step into this unit's loop -- so exp never waits.
            def emit_attention_unit(b, hf, h, prev_evac, prev_rest,
                                    hook=None):
                t0 = b * S
                qb0 = hf * QW                  # query base within batch
                q0 = t0 + qb0                  # query base global
                nkb = (qb0 + QW) // 128        # key blocks: 8 or 16
                d0 = h * 64

                def emit_scores(kb):
                    c_lo = max(0, 128 * kb - qb0)
                    s_ps = ps_s.tile([128, QW], F32, tag="s", name="s_ps")
                    for s0 in (0, 512):
                        lo = max(c_lo, s0)
                        if lo < s0 + 512:
                            nc.tensor.matmul(
                                s_ps[:, lo:s0 + 512],
                                kTp[h][:, t0 + kb * 128:t0 + (kb + 1) * 128],
                                qT_sb[:, q0 + lo:q0 + s0 + 512],
                                start=True, stop=True)
                    return s_ps, c_lo

                s_tiles = {0: emit_scores(0)}
                if nkb > 1:
                    s_tiles[1] = emit_scores(1)
                if prev_evac is not None:
                    prev_evac()
                cn_ps = ps_c.tile([128, QW], F32, tag="c", name="cn_ps")
                for kb in range(nkb):
                    if kb + 2 < nkb:
                        s_tiles[kb + 2] = emit_scores(kb + 2)
                    s_ps, c_lo = s_tiles.pop(kb)
                    w = QW - c_lo
                    e_sb = ep.tile([128, QW], BF16, tag="e", name="e_sb")
                    nc.scalar.activation(e_sb[:, 0:w], s_ps[:, c_lo:QW],
                                         AFT.Exp, scale=0.125)
                    dcol = 128 * kb - qb0
                    if dcol >= 0:  # diagonal block: in-block causal mask
                        nc.vector.tensor_mul(e_sb[:, 0:128],
                                             e_sb[:, 0:128], tri_sb[:])
                    for s0 in (0, 512):
                        lo = max(c_lo, s0)
                        if lo < s0 + 512:
                            nc.tensor.matmul(
                                cn_ps[0:66, lo:s0 + 512],
                                vN_sb[:, b * (S // 128) + kb,
                                      h * 66:(h + 1) * 66],
                                e_sb[:, lo - c_lo:s0 + 512 - c_lo],
                                start=(kb == 0), stop=(kb == nkb - 1),
                                skip_group_check=True)
                    if kb == 0 and prev_rest is not None:
                        prev_rest()
                    if kb == 1 and hook is not None:
                        hook()
                    maybe_fill()

                ct = rp.tile([128, QW], F32, tag="ct", name="ct")

                def evac():
                    # evacuate ctx+denominator to SBUF, freeing cn_ps; all
                    # pending deferred v transposes then ride the free slot
                    nc.vector.tensor_copy(ct[0:66, :], cn_ps[0:66, :])
                    while deferred_tr:
                        emit_v_transposes(*deferred_tr.pop(0))

                def rest():
                    # flip the denominator row into [128, 8], reciprocal at
                    # full lane parallelism, broadcast back via stride-0
                    # stationary matmuls against the fp32 identity, multiply.
                    T8 = ps_m.tile([128, 8], F32, tag="m", name="T8")
                    for j in range(8):
                        nc.tensor.transpose(
                            T8[:, j:j + 1], ct[64:65, j * 128:(j + 1) * 128],
                            onef[64:65, :])
                    R8 = rp.tile([128, 8], F32, tag="r8", name="R8")
                    nc.vector.reciprocal(R8[:], T8[:])
                    for half in range(2):
                        bc = ps_m.tile([64, 512], F32, tag="m", name="bc")
                        for jj in range(4):
                            j = half * 4 + jj
                            col = R8[:, j:j + 1]
                            lhsT = bass.AP(tensor=col.tensor, offset=col.offset,
                                           ap=[col.ap[0], [0, 64]])
                            nc.tensor.matmul(
                                bc[0:64, jj * 128:(jj + 1) * 128], lhsT,
                                idf_sb[:], start=True, stop=True)
                        sg = slice(half * 512, (half + 1) * 512)
                        nc.vector.tensor_mul(
                            ctxT_sb[d0:d0 + 64, q0 + half * 512:
                                    q0 + (half + 1) * 512],
                            ct[0:64, sg], bc[0:64, :])

                return evac, rest

            # ---- A2A + local Wo projection --------------------------------
            def emit_half_a2a_head(b, hf, h):
                # half-payload A2A carrying one head's 64 ctx rows; used to
                # overlap most of the final half's exchange with its last
                # attention unit
                base = b * S + hf * (S // 2)
                r0 = h * 64
                ctxd = dram.tile([N_CORES, 64, PH], BF16, tag=f"ctxdh{h}",
                                 name="ctxdh", bufs=1)
                for j in range(N_CORES):
                    nc.sync.dma_start(
                        out=ctxd[j],
                        in_=ctxT_sb[r0:r0 + 64,
                                    base + j * PH:base + (j + 1) * PH])
                recv = dram.tile([N_CORES, 64, PH], BF16, tag=f"recvh{h}",
                                 name="recvh", bufs=1)
                nc.gpsimd.collective_compute(
                    "AllToAll", mybir.AluOpType.bypass,
                    replica_groups=[list(range(N_CORES))],
                    ins=[ctxd.opt()], outs=[recv.opt()],
                )
                return recv

            def emit_half_a2a(b, hf):
                base = b * S + hf * (S // 2)
                ctxd = dram.tile([N_CORES, 128, PH], BF16, tag="ctxd",
                                 name="ctxd", bufs=4)
                for j in range(N_CORES):
                    nc.sync.dma_start(
                        out=ctxd[j],
                        in_=ctxT_sb[:, base + j * PH:base + (j + 1) * PH])
                recv = dram.tile([N_CORES, 128, PH], BF16, tag="recv",
                                 name="recv", bufs=4)
                nc.gpsimd.collective_compute(
                    "AllToAll",
                    mybir.AluOpType.bypass,
                    replica_groups=[list(range(N_CORES))],
                    ins=[ctxd.opt()],
                    outs=[recv.opt()],
                )
                return recv

            def emit_half_gather(b, hf, recv):
                cg_sb = op.tile([128, NKC, PH], BF16, tag="cg_sb", name="cg_sb")
                for j in range(N_CORES):
                    nc.sync.dma_start(out=cg_sb[:, j, :], in_=recv[j])
                return cg_sb

            def emit_half_proj(b, hf, cg_sb):
                o_sb = op.tile([PH, E], F32, tag="o_sb", name="o_sb")
                for et in range(2):
                    ps = ps_m.tile([128, 512], F32, tag="m", name="c_ps")
                    for kc in range(NKC):
                        nc.tensor.matmul(
                            ps[0:PH, :],
                            cg_sb[:, kc, :],
                            wo_sb[:, kc, et * 512:(et + 1) * 512],
                            start=(kc == 0), stop=(kc == NKC - 1))
                    nc.vector.tensor_add(
                        o_sb[:, et * 512:(et + 1) * 512], ps[0:PH, :],
                        bo_bc[0:PH, et * 512:(et + 1) * 512])
                r0 = (b * 2 + hf) * PH
                nc.sync.dma_start(out=out[r0:r0 + PH, :], in_=o_sb[:])

            # ---- main schedule --------------------------------------------
            # only tokens 0-1023 are needed before the first attention unit
            for tt in range(2):
                emit_qkv_tile(tt)

            sent = []      # (b, hf, recv): a2a issued, gather not emitted
            gathered = []  # (b, hf, cg_sb): gather emitted, proj not
            a2a_req = None  # (b, hf) whose ctx is complete, a2a not issued
            prev_evac = prev_rest = None
            # batch 1 processes hf1 before hf0 so the final (serial) A2A +
            # projection covers the cheap 8-key-block half.
            order = [(0, 0), (0, 1), (1, 1), (1, 0)]
            recvh0_box = []
            for b, hf in order:
                last_half = (b, hf) == order[-1]
                for h in range(2):
                    hook = None
                    if last_half and h == 1:
                        # fire the final half's head-0 A2A as soon as its
                        # normalization (run in this unit's prologue) lands
                        hook = (lambda: recvh0_box.append(
                            emit_half_a2a_head(b, hf, 0)))
                    prev_evac, prev_rest = emit_attention_unit(
                        b, hf, h, prev_evac, prev_rest, hook=hook)
                    # advance the a2a -> gather -> projection pipeline one
                    # stage per unit so each step overlaps attention compute
                    if gathered:
                        emit_half_proj(*gathered.pop(0))
                    if sent:
                        bb, hh2, recv = sent.pop(0)
                        gathered.append(
                            (bb, hh2, emit_half_gather(bb, hh2, recv)))
                    if a2a_req is not None:
                        # previous half's ctx is fully written (its norm ran
                        # inside this unit's prologue)
                        sent.append((a2a_req[0], a2a_req[1],
                                     emit_half_a2a(*a2a_req)))
                        a2a_req = None
                if not last_half:
                    a2a_req = (b, hf)
            prev_evac()
            prev_rest()
            while filler:
                tt, w, hh = filler.pop(0)
                box = fill_box.setdefault((tt, w), [None])
                vt_sb = emit_qkv_chain(tt, w, halves=(hh,), ps_box=box)
                if vt_sb is not None:
                    deferred_tr.append((tt, vt_sb))
            while deferred_tr:
                emit_v_transposes(*deferred_tr.pop(0))
            while sent:
                bb, hh2, recv = sent.pop(0)
                gathered.append((bb, hh2, emit_half_gather(bb, hh2, recv)))
            while gathered:
                emit_half_proj(*gathered.pop(0))
            # final half: head-1 A2A now, head-0 was fired mid-attention
            bL, hfL = order[-1]
            recvh1 = emit_half_a2a_head(bL, hfL, 1)
            cg_sb = op.tile([128, NKC, PH], BF16, tag="cg_sb", name="cg_sb")
            for j in range(N_CORES):
                nc.sync.dma_start(out=cg_sb[0:64, j, :], in_=recvh0_box[0][j])
                nc.sync.dma_start(out=cg_sb[64:128, j, :], in_=recvh1[j])
            emit_half_proj(bL, hfL, cg_sb)

    nc.compile()
    return nc


_NC = None


def _get_program():
    global _NC
    if _NC is None:
        _NC = build_program()
    return _NC


def _bf(a):
    return np.ascontiguousarray(a).astype(ml_dtypes.bfloat16)


def kernel(x, Wq, bq, Wk, bk, Wv, bv, Wo, bo, _trace=False, _trace_kwargs=None):
    x = np.asarray(x, np.float32)
    Wq, Wk, Wv, Wo = (np.asarray(w, np.float32) for w in (Wq, Wk, Wv, Wo))
    bq, bk, bv, bo = (np.asarray(v, np.float32) for v in (bq, bk, bv, bo))

    xT = _bf(x.reshape(T, E).T)
    i = np.arange(128)
    tri = _bf((i[:, None] <= i[None, :]).astype(np.float32))
    ident = _bf(np.eye(128, dtype=np.float32))

    def _packw(w_slice):
        # [DPC, E] weight slice -> [128, kc, 128]: p-major rows so the SBUF
        # load is one DMA with 2 KiB per-partition descriptors
        return _bf(np.ascontiguousarray(
            w_slice.T.reshape(NKC, 128, DPC).transpose(1, 0, 2)))

    in_maps = []
    for c in range(N_CORES):
        sl = slice(c * DPC, (c + 1) * DPC)
        in_maps.append({
            "xT": xT,
            "wq2": _packw(Wq[sl, :]),
            "wk2": _packw(Wk[sl, :]),
            "wv2": _packw(Wv[sl, :]),
            "woT": _bf(Wo.T),
            "bqkv": np.ascontiguousarray(
                np.stack([bq[sl], bk[sl], bv[sl]], axis=1).astype(np.float32)),
            "bo": bo,
            "tri": tri,
            "ident": ident,
        })

    nc = _get_program()
    res = run_bass_kernel_spmd(nc, in_maps, list(range(N_CORES)),
                               trace=_trace, **(_trace_kwargs or {}))
    # out[c] rows are [batch, half, 128]: row (b, hf, r) holds global
    # token b*2048 + hf*1024 + c*128 + r.
    stacked = np.stack([res.results[i]["out"].reshape(B, 2, 128, E)
                        for i in range(N_CORES)], axis=2)
    full = stacked.reshape(T, E)
    if _trace:
        return full.reshape(B, S, E), res
    return full.reshape(B, S, E)


# revision 23
# speedup vs baseline: 1.0637x; 1.0153x over previous
"""Multi-head attention (B=2, S=2048, H=16, D=64) on 8 Trainium2 NeuronCores.

Sharding: head-parallel tensor parallelism. Core c owns heads {2c, 2c+1}
(a 128-dim slice of the model dim): column-parallel QKV projections and
local causal attention for its 2 heads, AllToAll of normalized bf16
context (1 MiB/core) pipelined behind attention, then each core runs the
full-width Wo projection for its own disjoint 128-token slices.

Structure (what profiling drove):
 - Attention runs kb-major per (batch, half, head) unit: scores for one
   128-key block against 1024 queries land in a 2-bank PSUM tile, so ONE
   activation call exponentiates [128, <=1024] (96 wide calls instead of
   160 narrow ones; the 352-cycle ACT fixed cost was ~40% of exp time).
 - A*V is flipped: the stationary operand is v [128 keys, 64 dims + ones
   column] and the exp tile streams as the moving operand, so ctx
   accumulates as [dims, queries] in PSUM -- the layout the A2A and Wo
   projection want; no ctx transposes.  PSUM row 64 is the softmax
   denominator for free.
 - Scores run TWO key-blocks ahead of exp, and each unit's first two
   score matmuls are emitted before the previous unit's normalization
   chain, so the scalar engine's exp stream never waits on a unit
   boundary.
 - Normalization avoids the 1-lane DVE reciprocal (7.7 ns/elem on a
   [1,1024] row): the ctx tile is evacuated to SBUF (freeing its PSUM
   immediately), the denominator row is flipped into [128, 8] with 8
   tiny PE transposes, reciprocal'd at full lane parallelism, broadcast
   back with stride-0-stationary matmuls against an fp32 identity, and
   multiplied in.
 - DMA queues: weights lead on SP, x even chunks on SP, x odd chunks on
   the gpsimd software queue, tiny consts on the Activation queue (kept
   otherwise empty so exp is never queued behind a DMA trigger), wo on
   gpsimd behind a tiny warm-up AllToAll that absorbs the collective
   stream's first-op setup.  A2A staging/gathers/output all ride SP:
   software-DGE traffic was observed to triple AllToAll durations.
 - QKV for batch 1 is emitted as single projection chains interleaved
   between batch-0 attention steps; its v transposes (PE+identity) run
   at unit boundaries through the just-freed ctx PSUM slot.
"""

import sys

sys.path.insert(0, "/opt/trn_rl_repo")

import ml_dtypes
import numpy as np

import concourse.bass as bass
import concourse.tile as tile
from concourse import bacc, mybir
from concourse.bass_utils import run_bass_kernel_spmd

N_CORES = 8
B, S, H, D = 2, 2048, 16, 64
E = H * D            # 1024
T = B * S            # 4096 tokens
DPC = 128            # dims (2 heads) per core
NKC = E // 128       # 8 contraction chunks for the projections
NTT = T // 512       # 8 token tiles of 512
NTB = T // 128       # 32 token blocks of 128
PH = 128             # tokens per core per half-batch
QW = 1024            # queries per attention unit (= half batch)

F32 = mybir.dt.float32
BF16 = mybir.dt.bfloat16
AFT = mybir.ActivationFunctionType


def build_program():
    nc = bacc.Bacc("TRN2", target_bir_lowering=False, debug=False,
                   num_devices=N_CORES)

    xT = nc.dram_tensor("xT", [E, T], BF16, kind="ExternalInput").ap()
    # q/k/v weights pre-packed host-side as [sbuf partition, kc, col] so the
    # load is one DMA with 2 KiB descriptors (the [E, DPC] layout would give
    # 256 B descriptors -- measured ~10 GB/s and 35 us of prologue)
    wq2 = nc.dram_tensor("wq2", [128, NKC, DPC], BF16, kind="ExternalInput").ap()
    wk2 = nc.dram_tensor("wk2", [128, NKC, DPC], BF16, kind="ExternalInput").ap()
    wv2 = nc.dram_tensor("wv2", [128, NKC, DPC], BF16, kind="ExternalInput").ap()
    woT = nc.dram_tensor("woT", [E, E], BF16, kind="ExternalInput").ap()
    bqkv = nc.dram_tensor("bqkv", [DPC, 3], F32, kind="ExternalInput").ap()
    bo = nc.dram_tensor("bo", [E], F32, kind="ExternalInput").ap()
    # single 128x128 lower-triangular (k_local <= q_local) mask
    tri = nc.dram_tensor("tri", [128, 128], BF16, kind="ExternalInput").ap()
    ident = nc.dram_tensor("ident", [128, 128], BF16, kind="ExternalInput").ap()
    out = nc.dram_tensor("out", [T // N_CORES, E], F32, kind="ExternalOutput").ap()

    with tile.TileContext(nc) as tc:
        with (
            tc.tile_pool(name="consts", bufs=1) as consts,
            tc.tile_pool(name="state", bufs=1) as state,
            tc.tile_pool(name="ep", bufs=3) as ep,
            tc.tile_pool(name="rp", bufs=2) as rp,
            tc.tile_pool(name="op", bufs=2) as op,
            tc.tile_pool(name="ps_s", bufs=2, space="PSUM") as ps_s,
            tc.tile_pool(name="ps_c", bufs=1, space="PSUM") as ps_c,
            tc.tile_pool(name="ps_m", bufs=2, space="PSUM") as ps_m,
            tc.tile_pool(name="dram", bufs=1, space="DRAM") as dram,
        ):
            # ---- small constants first: the first QKV matmul needs wq,
            # so weights must not sit behind 8 MiB of x in the queues ----
            wq_sb = consts.tile([128, NKC, DPC], BF16)
            wk_sb = consts.tile([128, NKC, DPC], BF16)
            wv_sb = consts.tile([128, NKC, DPC], BF16)
            nc.sync.dma_start(out=wq_sb[:], in_=wq2[:])
            nc.sync.dma_start(out=wk_sb[:], in_=wk2[:])
            nc.sync.dma_start(out=wv_sb[:], in_=wv2[:])
            bqkv_sb = consts.tile([128, 3], F32)
            nc.scalar.dma_start(out=bqkv_sb[:], in_=bqkv[:])
            bq_sb = bqkv_sb[:, 0:1]
            bk_sb = bqkv_sb[:, 1:2]
            bv_sb = bqkv_sb[:, 2:3]
            bo_bc = consts.tile([128, E], F32)
            nc.scalar.dma_start(
                out=bo_bc[:],
                in_=bass.AP(tensor=bo.tensor, offset=bo.offset,
                            ap=[[0, 128], [1, E]]),
            )
            tri_sb = consts.tile([128, 128], BF16)
            nc.scalar.dma_start(out=tri_sb[:], in_=tri[:])
            id_sb = consts.tile([128, 128], BF16)
            nc.scalar.dma_start(out=id_sb[:], in_=ident[:])
            idf_sb = consts.tile([128, 128], F32)
            nc.vector.tensor_copy(idf_sb[:], id_sb[:])
            onef = consts.tile([128, 1], F32)
            nc.vector.memset(onef[:], 1.0)

            # tiny warm-up AllToAll on the gpsimd queue: absorbs the
            # collective stream's first-op setup while QKV runs
            ctxw = dram.tile([N_CORES, 128, 2], BF16, tag="ctxw", name="ctxw")
            recvw = dram.tile([N_CORES, 128, 2], BF16, tag="recvw",
                              name="recvw")
            nc.gpsimd.dma_start(out=ctxw[:], in_=tri[:, 0:16])
            nc.gpsimd.collective_compute(
                "AllToAll", mybir.AluOpType.bypass,
                replica_groups=[list(range(N_CORES))],
                ins=[ctxw.opt()], outs=[recvw.opt()],
            )

            # ---- x: a small fast-start chunk per kc so QKV tile 0 can
            # begin, then wide descriptors for bandwidth; batch-1 halves
            # alternate between SP and the gpsimd software queue ----
            x_sb = state.tile([128, NKC, T], BF16)  # full x^T in SBUF

            def x_load(eng, kc, c0, c1):
                eng.dma_start(out=x_sb[:, kc, c0:c1],
                              in_=xT[kc * 128:(kc + 1) * 128, c0:c1])

            # the gpsimd software queue measured ~3x the per-queue rate of
            # the two hardware DGE queues, so it takes half the stream
            engs4 = [nc.gpsimd, nc.sync, nc.gpsimd, nc.scalar]
            for kc in range(NKC):
                x_load(engs4[kc % 4], kc, 0, 1024)
            for kc in range(NKC):
                x_load(engs4[(kc + 1) % 4], kc, 1024, 2048)
            for kc in range(NKC):
                x_load(engs4[(kc + 2) % 4], kc, 2048, 3072)
            for kc in range(NKC):
                x_load(engs4[(kc + 3) % 4], kc, 3072, 4096)

            # wo on the gpsimd software queue behind x-odds
            wo_sb = consts.tile([128, NKC, E], BF16)
            for kc in range(NKC):
                nc.gpsimd.dma_start(out=wo_sb[:, kc, :],
                                    in_=woT[kc * 128:(kc + 1) * 128, :])

            # ---- persistent activations -----------------------------------
            qT_sb = state.tile([128, T], BF16)   # [2-head dims, tokens]
            # per-head k^T zero-padded to the full 128 partitions: head h
            # occupies partitions [64h, 64h+64), the rest stay zero.
            kTp = [state.tile([128, T], BF16, name=f"kTp{h}") for h in range(2)]
            # v in [token, dim] layout per 128-token block:
            # cols 0:64 = head0 v, 64:66 = [1, 0], 66:130 = head1 v,
            # 130:132 = [1, 0]. The [1,0] columns give each head's AV
            # stationary slice (0:66 / 66:132) a softmax-denominator row.
            vN_sb = state.tile([128, NTB, 132], BF16)
            ctxT_sb = state.tile([128, T], BF16)  # normalized ctx, [dims, tok]

            nc.vector.memset(kTp[0][64:128, :], 0.0)
            nc.vector.memset(kTp[1][0:64, :], 0.0)
            nc.vector.memset(vN_sb[:, :, 64:65], 1.0)
            nc.vector.memset(vN_sb[:, :, 65:66], 0.0)
            nc.vector.memset(vN_sb[:, :, 130:131], 1.0)
            nc.vector.memset(vN_sb[:, :, 131:132], 0.0)

            # ---- QKV projection emitters ----------------------------------
            # epilogues on DVE; v's [token, dim] reshape via PE transposes
            # whose PSUM rides the ps_c slot (free between attention units).
            def emit_v_transpose_one(tt, vt_sb, i):
                tb = tt * 4 + i
                tp_ps = ps_c.tile([128, 128], BF16, tag="c", name="tp_ps")
                nc.tensor.transpose(
                    tp_ps[:], vt_sb[:, i * 128:(i + 1) * 128], id_sb[:])
                nc.vector.tensor_copy(vN_sb[:, tb, 0:64], tp_ps[:, 0:64])
                nc.vector.tensor_copy(vN_sb[:, tb, 66:130], tp_ps[:, 64:128])

            def emit_v_transposes(tt, vt_sb):
                for i in range(4):
                    emit_v_transpose_one(tt, vt_sb, i)

            def emit_qkv_chain(tt, which, halves=(0, 1), ps_box=[None]):
                ts = slice(tt * 512, (tt + 1) * 512)
                w_sb, b_sb = {"q": (wq_sb, bq_sb), "k": (wk_sb, bk_sb),
                              "v": (wv_sb, bv_sb)}[which]
                if 0 in halves:
                    ps_box[0] = ps_m.tile([128, 512], F32, tag="m", name="ps")
                ps = ps_box[0]
                for hh in halves:
                    for kc in range(hh * 4, hh * 4 + 4):
                        nc.tensor.matmul(ps[:], w_sb[:, kc, :],
                                         x_sb[:, kc, ts],
                                         start=(kc == 0),
                                         stop=(kc == NKC - 1))
                if 1 not in halves:
                    return None
                if which == "q":
                    nc.vector.tensor_scalar_add(qT_sb[:, ts], ps[:], b_sb[:])
                elif which == "k":
                    nc.vector.tensor_scalar_add(
                        kTp[0][0:64, ts], ps[0:64, :], b_sb[0:64, :])
                    nc.vector.tensor_scalar_add(
                        kTp[1][64:128, ts], ps[64:128, :], b_sb[64:128, :])
                else:
                    vt_sb = rp.tile([128, 512], BF16, tag="vt", name="vt",
                                    bufs=4)
                    nc.vector.tensor_scalar_add(vt_sb[:], ps[:], b_sb[:])
                    return vt_sb
                return None

            def emit_qkv_tile(tt):
                emit_qkv_chain(tt, "q")
                emit_qkv_chain(tt, "k")
                vt_sb = emit_qkv_chain(tt, "v")
                emit_v_transposes(tt, vt_sb)

            # deferred QKV work (tiles 2..7): half-chains between attention
            # steps; v transposes wait for a unit boundary where the freed
            # ctx PSUM slot can host them (tiles 2-3 feed the second
            # batch-0 half, 4-7 must all be in place before batch 1)
            filler = [(tt, w, hh) for tt in range(2, NTT)
                      for w in ("q", "k", "v") for hh in (0, 1)]
            fill_box = {}
            deferred_tr = []
            step_count = [0]

            def maybe_fill():
                # skip the first 8 attention steps: x for tiles 2+ is still
                # in flight then, and a stalled filler matmul would block
                # the whole PE queue (strict FIFO) and starve exp
                step_count[0] += 1
                if filler and step_count[0] > 8:
                    tt, w, hh = filler.pop(0)
                    box = fill_box.setdefault((tt, w), [None])
                    vt_sb = emit_qkv_chain(tt, w, halves=(hh,), ps_box=box)
                    if vt_sb is not None:
                        deferred_tr.append((tt, vt_sb))

            # ---- attention unit: (batch b, half hf, head h) ---------------
            # kb-major: scores for each 128-key block land in a [128, 1024]
            # PSUM tile (two ahead of exp), one exp call per block, A*V
            # accumulates ctx [66, 1024].  The previous unit's finisher is
            # split in two: evac (frees its ctx PSUM slot; must precede this
            # unit's ctx allocation) runs after this unit's first two score
            # matmuls, and the reciprocal/broadcast/normalize chain runs one
            # kb-step into this unit's loop -- so exp never waits.
            def emit_attention_unit(b, hf, h, prev_evac, prev_rest,
                                    hook=None):
                t0 = b * S
                qb0 = hf * QW                  # query base within batch
                q0 = t0 + qb0                  # query base global
                nkb = (qb0 + QW) // 128        # key blocks: 8 or 16
                d0 = h * 64

                def emit_scores(kb):
                    c_lo = max(0, 128 * kb - qb0)
                    s_ps = ps_s.tile([128, QW], F32, tag="s", name="s_ps")
                    for s0 in (0, 512):
                        lo = max(c_lo, s0)
                        if lo < s0 + 512:
                            nc.tensor.matmul(
                                s_ps[:, lo:s0 + 512],
                                kTp[h][:, t0 + kb * 128:t0 + (kb + 1) * 128],
                                qT_sb[:, q0 + lo:q0 + s0 + 512],
                                start=True, stop=True)
                    return s_ps, c_lo

                s_tiles = {0: emit_scores(0)}
                if nkb > 1:
                    s_tiles[1] = emit_scores(1)
                if prev_evac is not None:
                    prev_evac()
                cn_ps = ps_c.tile([128, QW], F32, tag="c", name="cn_ps")
                for kb in range(nkb):
                    if kb + 2 < nkb:
                        s_tiles[kb + 2] = emit_scores(kb + 2)
                    s_ps, c_lo = s_tiles.pop(kb)
                    w = QW - c_lo
                    e_sb = ep.tile([128, QW], BF16, tag="e", name="e_sb")
                    nc.scalar.activation(e_sb[:, 0:w], s_ps[:, c_lo:QW],
                                         AFT.Exp, scale=0.125)
                    dcol = 128 * kb - qb0
                    if dcol >= 0:  # diagonal block: in-block causal mask
                        nc.vector.tensor_mul(e_sb[:, 0:128],
                                             e_sb[:, 0:128], tri_sb[:])
                    for s0 in (0, 512):
                        lo = max(c_lo, s0)
                        if lo < s0 + 512:
                            nc.tensor.matmul(
                                cn_ps[0:66, lo:s0 + 512],
                                vN_sb[:, b * (S // 128) + kb,
                                      h * 66:(h + 1) * 66],
                                e_sb[:, lo - c_lo:s0 + 512 - c_lo],
                                start=(kb == 0), stop=(kb == nkb - 1),
                                skip_group_check=True)
                    if kb == 0 and prev_rest is not None:
                        prev_rest()
                    if kb == 1 and hook is not None:
                        hook()
                    maybe_fill()

                ct = rp.tile([128, QW], F32, tag="ct", name="ct")

                def evac():
                    # evacuate ctx+denominator to SBUF, freeing cn_ps; all
                    # pending deferred v transposes then ride the free slot
                    nc.vector.tensor_copy(ct[0:66, :], cn_ps[0:66, :])
                    while deferred_tr:
                        emit_v_transposes(*deferred_tr.pop(0))

                def rest():
                    # flip the denominator row into [128, 8], reciprocal at
                    # full lane parallelism, broadcast back via stride-0
                    # stationary matmuls against the fp32 identity, multiply.
                    T8 = ps_m.tile([128, 8], F32, tag="m", name="T8")
                    for j in range(8):
                        nc.tensor.transpose(
                            T8[:, j:j + 1], ct[64:65, j * 128:(j + 1) * 128],
                            onef[64:65, :])
                    R8 = rp.tile([128, 8], F32, tag="r8", name="R8")
                    nc.vector.reciprocal(R8[:], T8[:])
                    for half in range(2):
                        bc = ps_m.tile([64, 512], F32, tag="m", name="bc")
                        for jj in range(4):
                            j = half * 4 + jj
                            col = R8[:, j:j + 1]
                            lhsT = bass.AP(tensor=col.tensor, offset=col.offset,
                                           ap=[col.ap[0], [0, 64]])
                            nc.tensor.matmul(
                                bc[0:64, jj * 128:(jj + 1) * 128], lhsT,
                                idf_sb[:], start=True, stop=True)
                        sg = slice(half * 512, (half + 1) * 512)
                        nc.vector.tensor_mul(
                            ctxT_sb[d0:d0 + 64, q0 + half * 512:
                                    q0 + (half + 1) * 512],
                            ct[0:64, sg], bc[0:64, :])

                return evac, rest

            # ---- A2A + local Wo projection --------------------------------
            def emit_half_a2a_head(b, hf, h):
                # half-payload A2A carrying one head's 64 ctx rows; used to
                # overlap most of the final half's exchange with its last
                # attention unit
                base = b * S + hf * (S // 2)
                r0 = h * 64
                ctxd = dram.tile([N_CORES, 64, PH], BF16, tag=f"ctxdh{h}",
                                 name="ctxdh", bufs=1)
                for j in range(N_CORES):
                    nc.sync.dma_start(
                        out=ctxd[j],
                        in_=ctxT_sb[r0:r0 + 64,
                                    base + j * PH:base + (j + 1) * PH])
                recv = dram.tile([N_CORES, 64, PH], BF16, tag=f"recvh{h}",
                                 name="recvh", bufs=1)
                nc.gpsimd.collective_compute(
                    "AllToAll", mybir.AluOpType.bypass,
                    replica_groups=[list(range(N_CORES))],
                    ins=[ctxd.opt()], outs=[recv.opt()],
                )
                return recv

            def emit_half_a2a(b, hf):
                base = b * S + hf * (S // 2)
                ctxd = dram.tile([N_CORES, 128, PH], BF16, tag="ctxd",
                                 name="ctxd", bufs=4)
                for j in range(N_CORES):
                    nc.sync.dma_start(
                        out=ctxd[j],
                        in_=ctxT_sb[:, base + j * PH:base + (j + 1) * PH])
                recv = dram.tile([N_CORES, 128, PH], BF16, tag="recv",
                                 name="recv", bufs=4)
                nc.gpsimd.collective_compute(
                    "AllToAll",
                    mybir.AluOpType.bypass,
                    replica_groups=[list(range(N_CORES))],
                    ins=[ctxd.opt()],
                    outs=[recv.opt()],
                )
                return recv

            def emit_half_gather(b, hf, recv):
                cg_sb = op.tile([128, NKC, PH], BF16, tag="cg_sb", name="cg_sb")
                for j in range(N_CORES):
                    nc.sync.dma_start(out=cg_sb[:, j, :], in_=recv[j])
                return cg_sb

            def emit_half_proj(b, hf, cg_sb):
                o_sb = op.tile([PH, E], F32, tag="o_sb", name="o_sb")
                for et in range(2):
                    ps = ps_m.tile([128, 512], F32, tag="m", name="c_ps")
                    for kc in range(NKC):
                        nc.tensor.matmul(
                            ps[0:PH, :],
                            cg_sb[:, kc, :],
                            wo_sb[:, kc, et * 512:(et + 1) * 512],
                            start=(kc == 0), stop=(kc == NKC - 1))
                    nc.vector.tensor_add(
                        o_sb[:, et * 512:(et + 1) * 512], ps[0:PH, :],
                        bo_bc[0:PH, et * 512:(et + 1) * 512])
                r0 = (b * 2 + hf) * PH
                nc.sync.dma_start(out=out[r0:r0 + 64, :], in_=o_sb[0:64, :])
                nc.scalar.dma_start(out=out[r0 + 64:r0 + PH, :],
                                    in_=o_sb[64:128, :])

            # ---- main schedule --------------------------------------------
            # only tokens 0-1023 are needed before the first attention unit
            for tt in range(2):
                emit_qkv_tile(tt)

            sent = []      # (b, hf, recv): a2a issued, gather not emitted
            gathered = []  # (b, hf, cg_sb): gather emitted, proj not
            a2a_req = None  # (b, hf) whose ctx is complete, a2a not issued
            prev_evac = prev_rest = None
            # batch 1 processes hf1 before hf0 so the final (serial) A2A +
            # projection covers the cheap 8-key-block half.
            order = [(0, 0), (0, 1), (1, 1), (1, 0)]
            recvh0_box = []
            for b, hf in order:
                last_half = (b, hf) == order[-1]
                for h in range(2):
                    hook = None
                    if last_half and h == 1:
                        # fire the final half's head-0 A2A as soon as its
                        # normalization (run in this unit's prologue) lands
                        hook = (lambda: recvh0_box.append(
                            emit_half_a2a_head(b, hf, 0)))
                    prev_evac, prev_rest = emit_attention_unit(
                        b, hf, h, prev_evac, prev_rest, hook=hook)
                    # advance the a2a -> gather -> projection pipeline one
                    # stage per unit so each step overlaps attention compute
                    if gathered:
                        emit_half_proj(*gathered.pop(0))
                    if sent:
                        bb, hh2, recv = sent.pop(0)
                        gathered.append(
                            (bb, hh2, emit_half_gather(bb, hh2, recv)))
                    if a2a_req is not None:
                        # previous half's ctx is fully written (its norm ran
                        # inside this unit's prologue)
                        sent.append((a2a_req[0], a2a_req[1],
                                     emit_half_a2a(*a2a_req)))
                        a2a_req = None
                if not last_half:
                    a2a_req = (b, hf)
            prev_evac()
            prev_rest()
            while filler:
                tt, w, hh = filler.pop(0)
                box = fill_box.setdefault((tt, w), [None])
                vt_sb = emit_qkv_chain(tt, w, halves=(hh,), ps_box=box)
                if vt_sb is not None:
                    deferred_tr.append((tt, vt_sb))
            while deferred_tr:
                emit_v_transposes(*deferred_tr.pop(0))
            while sent:
                bb, hh2, recv = sent.pop(0)
                gathered.append((bb, hh2, emit_half_gather(bb, hh2, recv)))
            while gathered:
                emit_half_proj(*gathered.pop(0))
            # final half: head-1 A2A now, head-0 was fired mid-attention
            bL, hfL = order[-1]
            recvh1 = emit_half_a2a_head(bL, hfL, 1)
            cg_sb = op.tile([128, NKC, PH], BF16, tag="cg_sb", name="cg_sb")
            for j in range(N_CORES):
                nc.sync.dma_start(out=cg_sb[0:64, j, :], in_=recvh0_box[0][j])
                nc.sync.dma_start(out=cg_sb[64:128, j, :], in_=recvh1[j])
            emit_half_proj(bL, hfL, cg_sb)

    nc.compile()
    return nc


_NC = None


def _get_program():
    global _NC
    if _NC is None:
        _NC = build_program()
    return _NC


def _bf(a):
    return np.ascontiguousarray(a).astype(ml_dtypes.bfloat16)


def kernel(x, Wq, bq, Wk, bk, Wv, bv, Wo, bo, _trace=False, _trace_kwargs=None):
    x = np.asarray(x, np.float32)
    Wq, Wk, Wv, Wo = (np.asarray(w, np.float32) for w in (Wq, Wk, Wv, Wo))
    bq, bk, bv, bo = (np.asarray(v, np.float32) for v in (bq, bk, bv, bo))

    xT = _bf(x.reshape(T, E).T)
    i = np.arange(128)
    tri = _bf((i[:, None] <= i[None, :]).astype(np.float32))
    ident = _bf(np.eye(128, dtype=np.float32))

    def _packw(w_slice):
        # [DPC, E] weight slice -> [128, kc, 128]: p-major rows so the SBUF
        # load is one DMA with 2 KiB per-partition descriptors
        return _bf(np.ascontiguousarray(
            w_slice.T.reshape(NKC, 128, DPC).transpose(1, 0, 2)))

    in_maps = []
    for c in range(N_CORES):
        sl = slice(c * DPC, (c + 1) * DPC)
        in_maps.append({
            "xT": xT,
            "wq2": _packw(Wq[sl, :]),
            "wk2": _packw(Wk[sl, :]),
            "wv2": _packw(Wv[sl, :]),
            "woT": _bf(Wo.T),
            "bqkv": np.ascontiguousarray(
                np.stack([bq[sl], bk[sl], bv[sl]], axis=1).astype(np.float32)),
            "bo": bo,
            "tri": tri,
            "ident": ident,
        })

    nc = _get_program()
    res = run_bass_kernel_spmd(nc, in_maps, list(range(N_CORES)),
                               trace=_trace, **(_trace_kwargs or {}))
    # out[c] rows are [batch, half, 128]: row (b, hf, r) holds global
    # token b*2048 + hf*1024 + c*128 + r.
    stacked = np.stack([res.results[i]["out"].reshape(B, 2, 128, E)
                        for i in range(N_CORES)], axis=2)
    full = stacked.reshape(T, E)
    if _trace:
        return full.reshape(B, S, E), res
    return full.reshape(B, S, E)
